# revision 2
# baseline (speedup 1.0000x reference)
"""CrossMamba Trainium2 kernel, v2.

Sharding: 8 cores = 4 batches x 2 scan directions (pure data parallel, no
collectives). Backward direction handled by host-side time flip; every core
runs the same SPMD program.

v2 redesign vs v1:
  - c_in folded into in_proj on the host (W1 = in_proj_w @ c_in_w for the
    context segment, in_proj_w for the query segment; segment bias applied
    at the PSUM->SBUF copy). Phase A eliminated.
  - z-projection fused into the same weight stream as u.
  - GEMM weights/activations in bf16 (same PE rate, half the DMA/SBUF).
  - delta computed inside phase F per time-half (no delta/dg spills).
  - Selective scan: scans on Pool (gpsimd), dgB/ch multiplies on DVE,
    exp(A*delta) on ACT, state accumulation via identity matmuls into PSUM
    on PE. Scan state carried across time-halves through a PSUM tile.
  - out_proj accumulated from PSUM per time-half (no yg spill).
"""
import numpy as np

B, Lq, Lc = 4, 1024, 1024
DQ, DC, DM = 1024, 768, 1024
DS, DCONV = 16, 4
DI, DTR = 2048, 64
L = Lc + Lq              # 2048
NCORE = 8
NE = DI // 128           # 16 channel blocks
NK = DM // 128           # 8 contraction blocks
NT = L // 512            # 4 time blocks of 512
TBH = 1024               # time half

_prog = None


def _build():
    import concourse.bacc as bacc
    import concourse.tile as tile
    from concourse import mybir

    f32 = mybir.dt.float32
    bf16 = mybir.dt.bfloat16
    f16 = mybir.dt.float16
    MUL = mybir.AluOpType.mult
    ADD = mybir.AluOpType.add
    AF = mybir.ActivationFunctionType

    nc = bacc.Bacc("TRN2", target_bir_lowering=False, debug=False,
                   num_devices=NCORE)

    # ---- per-core external inputs ----
    in1_d = [nc.dram_tensor(f"in1_{s}", [DM, TBH], bf16, kind="ExternalInput")
             for s in range(2)]
    W1_d = nc.dram_tensor("W1", [2, 2 * NE, 128, DM], bf16,
                          kind="ExternalInput")
    biasU_d = nc.dram_tensor("biasU", [128, 2 * NE], f32, kind="ExternalInput")
    biasZ_d = nc.dram_tensor("biasZ", [128, 2 * NE], f32, kind="ExternalInput")
    Wxp_d = nc.dram_tensor("Wxp", [128, NE * 96], bf16, kind="ExternalInput")
    Wdt_d = nc.dram_tensor("Wdt", [DTR, DI], bf16, kind="ExternalInput")
    Wout_d = nc.dram_tensor("Wout", [8, 128, NE * 128], bf16,
                            kind="ExternalInput")
    convw_d = nc.dram_tensor("convw", [128, NE * DCONV], f32,
                             kind="ExternalInput")
    convb_d = nc.dram_tensor("convb", [128, NE], f32, kind="ExternalInput")
    dtb_d = nc.dram_tensor("dtb", [128, NE], f32, kind="ExternalInput")
    Ah_d = nc.dram_tensor("Ah", [128, NE * DS], f32, kind="ExternalInput")
    Dd_d = nc.dram_tensor("Dd", [128, NE * 128], bf16, kind="ExternalInput")
    Ident_d = nc.dram_tensor("Ident", [128, 128], bf16, kind="ExternalInput")

    # ---- DRAM scratch ----
    u_sp = nc.dram_tensor("u_sp", [DI, L], bf16)
    zs_sp = nc.dram_tensor("zs_sp", [DI, L], bf16)
    bc_sp = nc.dram_tensor("bc_sp", [2 * DS, L], bf16)

    out_d = nc.dram_tensor("out", [DM, L], f32, kind="ExternalOutput")

    with tile.TileContext(nc) as tc:
        with (
            tc.tile_pool(name="wp", bufs=1) as wp,
        ):
            # small persistent weights
            convw = wp.tile([128, NE * DCONV], f32, tag="convw")
            nc.sync.dma_start(convw[:], convw_d[:])
            convb = wp.tile([128, NE], f32, tag="convb")
            nc.sync.dma_start(convb[:], convb_d[:])
            dtb = wp.tile([128, NE], f32, tag="dtb")
            nc.sync.dma_start(dtb[:], dtb_d[:])
            Ah = wp.tile([128, NE * DS], f32, tag="Ah")
            nc.sync.dma_start(Ah[:], Ah_d[:])
            Dd = wp.tile([128, NE * 128], bf16, tag="Dd")
            nc.sync.dma_start(Dd[:], Dd_d[:])
            Ident = wp.tile([128, 128], bf16, tag="Ident")
            nc.sync.dma_start(Ident[:], Ident_d[:])
            biasU = wp.tile([128, 2 * NE], f32, tag="biasU")
            nc.sync.dma_start(biasU[:], biasU_d[:])
            biasZ = wp.tile([128, 2 * NE], f32, tag="biasZ")
            nc.sync.dma_start(biasZ[:], biasZ_d[:])

            # carries: [128, e*16+s] f32, lives across both time halves
            # (SBUF: GPSIMD scan cannot read a PSUM initial)
            carry = wp.tile([128, NE * DS], f32, tag="carry")

            with tc.tile_pool(name="mid", bufs=1) as mid:
                Wxp = mid.tile([128, NE * 96], bf16, tag="Wxp")
                nc.sync.dma_start(Wxp[:], Wxp_d[:])
                Wdt = mid.tile([DTR, DI], bf16, tag="Wdt")
                nc.sync.dma_start(Wdt[:], Wdt_d[:])
                dt_r = mid.tile([DTR, L], bf16, tag="dt_r")
                bc = mid.tile([2 * DS, L], bf16, tag="bc")

                # ================= phase B: in_proj/conv/x_proj/z =========
                with (tc.tile_pool(name="pb", bufs=1) as pb,
                      tc.tile_pool(name="psb", bufs=2, space="PSUM") as psb,
                      tc.tile_pool(name="psxp", bufs=1, space="PSUM") as psxp):
                    in1 = []
                    for s in range(2):
                        row = []
                        for kb in range(NK):
                            t = pb.tile([128, TBH], bf16, tag=f"in1_{s}{kb}",
                                        name=f"in1_{s}{kb}")
                            nc.sync.dma_start(
                                t[:], in1_d[s][kb * 128:(kb + 1) * 128, :])
                            row.append(t)
                        in1.append(row)
                    xp_acc = [psxp.tile([96, 512], f32, tag=f"xp{tb}",
                                        name=f"xp{tb}") for tb in range(NT)]
                    for e in range(NE):
                        wtu = pb.tile([128, 2 * DM], bf16, tag="wtu", bufs=2)
                        nc.sync.dma_start(wtu[:, 0:DM], W1_d[0, e, :, :])
                        nc.sync.dma_start(wtu[:, DM:], W1_d[1, e, :, :])
                        wtz = pb.tile([128, 2 * DM], bf16, tag="wtz", bufs=2)
                        nc.sync.dma_start(wtz[:, 0:DM], W1_d[0, NE + e, :, :])
                        nc.sync.dma_start(wtz[:, DM:], W1_d[1, NE + e, :, :])
                        upre = pb.tile([128, L + 3], bf16, tag="upre", bufs=2)
                        nc.gpsimd.memset(upre[:, 0:3], 0.0)
                        for tb in range(NT):
                            seg = 0 if tb < 2 else 1
                            tloc = (tb % 2) * 512
                            acc = psb.tile([128, 512], f32, tag="pp")
                            for kb in range(NK):
                                nc.tensor.matmul(
                                    acc[:],
                                    wtu[:, seg * DM + kb * 128:
                                        seg * DM + (kb + 1) * 128],
                                    in1[seg][kb][:, tloc:tloc + 512],
                                    start=(kb == 0), stop=(kb == NK - 1))
                            nc.scalar.add(
                                upre[:, 3 + tb * 512: 3 + (tb + 1) * 512],
                                acc[:],
                                biasU[:, seg * NE + e: seg * NE + e + 1])
                        # depthwise causal conv: 4 products (DVE) + 3 adds
                        pk = []
                        for k in range(DCONV):
                            t = pb.tile([128, L], bf16, tag=f"cp{k}",
                                        name=f"cp{k}")
                            nc.vector.tensor_scalar(
                                out=t[:], in0=upre[:, k:k + L],
                                scalar1=convw[:, e * DCONV + k:
                                              e * DCONV + k + 1],
                                scalar2=None, op0=MUL)
                            pk.append(t)
                        a01 = pb.tile([128, L], bf16, tag="ca0", bufs=2)
                        nc.gpsimd.tensor_tensor(
                            out=a01[:], in0=pk[0][:], in1=pk[1][:], op=ADD)
                        a23 = pb.tile([128, L], bf16, tag="ca1", bufs=2)
                        nc.gpsimd.tensor_tensor(
                            out=a23[:], in0=pk[2][:], in1=pk[3][:], op=ADD)
                        asum = pb.tile([128, L], bf16, tag="ca2", bufs=2)
                        nc.gpsimd.tensor_tensor(
                            out=asum[:], in0=a01[:], in1=a23[:], op=ADD)
                        usilu = pb.tile([128, L], bf16, tag="usilu", bufs=2)
                        nc.scalar.activation(usilu[:], asum[:], AF.Silu,
                                             bias=convb[:, e:e + 1])
                        nc.sync.dma_start(
                            u_sp[e * 128:(e + 1) * 128, :], usilu[:])
                        for tb in range(NT):
                            nc.tensor.matmul(
                                xp_acc[tb][:],
                                Wxp[:, e * 96:(e + 1) * 96],
                                usilu[:, tb * 512:(tb + 1) * 512],
                                start=(e == 0), stop=(e == NE - 1))
                        # z half
                        zfull = pb.tile([128, L], bf16, tag="zfull", bufs=2)
                        for tb in range(NT):
                            seg = 0 if tb < 2 else 1
                            tloc = (tb % 2) * 512
                            acc = psb.tile([128, 512], f32, tag="pp")
                            for kb in range(NK):
                                nc.tensor.matmul(
                                    acc[:],
                                    wtz[:, seg * DM + kb * 128:
                                        seg * DM + (kb + 1) * 128],
                                    in1[seg][kb][:, tloc:tloc + 512],
                                    start=(kb == 0), stop=(kb == NK - 1))
                            nc.scalar.activation(
                                zfull[:, tb * 512:(tb + 1) * 512], acc[:],
                                AF.Silu,
                                bias=biasZ[:, seg * NE + e: seg * NE + e + 1])
                        nc.sync.dma_start(
                            zs_sp[e * 128:(e + 1) * 128, :], zfull[:])

                    # xp epilogue: dt rows + B/C rows
                    for tb in range(NT):
                        nc.scalar.copy(dt_r[:, tb * 512:(tb + 1) * 512],
                                       xp_acc[tb][0:DTR, :])
                        nc.scalar.copy(bc[:, tb * 512:(tb + 1) * 512],
                                       xp_acc[tb][DTR:96, :])
                    nc.sync.dma_start(bc_sp[:], bc[:])

                # ================= phase F: scan + gate + out_proj ========
                with (tc.tile_pool(name="pf", bufs=1) as pf,
                      tc.tile_pool(name="psy", bufs=2, space="PSUM") as psy,
                      tc.tile_pool(name="psd", bufs=2, space="PSUM") as psd,
                      tc.tile_pool(name="pso", bufs=1, space="PSUM") as pso):
                    for tbh in range(2):
                        hb = tbh * TBH
                        Bb, Cb = [], []
                        for s in range(DS):
                            bb = pf.tile([128, TBH], bf16, tag=f"Bb{s}",
                                         name=f"Bb{s}")
                            nc.sync.dma_start(
                                bb[:],
                                bc_sp[s:s + 1, hb:hb + TBH]
                                .partition_broadcast(128))
                            cb = pf.tile([128, TBH], bf16, tag=f"Cb{s}",
                                         name=f"Cb{s}")
                            nc.sync.dma_start(
                                cb[:],
                                bc_sp[DS + s:DS + s + 1, hb:hb + TBH]
                                .partition_broadcast(128))
                            Bb.append(bb)
                            Cb.append(cb)
                        yg = []
                        for e in range(NE):
                            # delta for this (e, half)
                            delta = pf.tile([128, TBH], f16, tag="dl", bufs=2)
                            for j in range(2):
                                accd = psd.tile([128, 512], f32, tag="dtp")
                                nc.tensor.matmul(
                                    accd[:], Wdt[:, e * 128:(e + 1) * 128],
                                    dt_r[:, hb + j * 512: hb + (j + 1) * 512],
                                    start=True, stop=True)
                                # softplus(x+b) = ln(1+exp(x+b)); |x+b| < 6
                                # here so exp cannot overflow
                                ex = pf.tile([128, 512], f32, tag="spexp",
                                             bufs=2)
                                nc.scalar.activation(
                                    ex[:], accd[:], AF.Exp,
                                    bias=dtb[:, e:e + 1])
                                nc.scalar.activation(
                                    delta[:, j * 512:(j + 1) * 512], ex[:],
                                    AF.Ln, bias=1.0)
                            ub = pf.tile([128, TBH], bf16, tag="ub", bufs=2)
                            nc.sync.dma_start(
                                ub[:], u_sp[e * 128:(e + 1) * 128,
                                            hb:hb + TBH])
                            zst = pf.tile([128, TBH], bf16, tag="zst", bufs=2)
                            nc.sync.dma_start(
                                zst[:], zs_sp[e * 128:(e + 1) * 128,
                                              hb:hb + TBH])
                            dg = pf.tile([128, TBH], bf16, tag="dg", bufs=2)
                            nc.vector.tensor_tensor(
                                out=dg[:], in0=delta[:], in1=ub[:], op=MUL)
                            yps = [psy.tile([128, 512], f32, tag=f"y{j}",
                                            name=f"y{j}") for j in range(2)]
                            for j in range(2):
                                nc.tensor.matmul(
                                    yps[j][:], Dd[:, e * 128:(e + 1) * 128],
                                    ub[:, j * 512:(j + 1) * 512],
                                    start=True, stop=False,
                                    skip_group_check=True)
                            for s in range(DS):
                                idx = e * DS + s
                                dA = pf.tile([128, TBH], f16, tag="dA",
                                             bufs=2)
                                nc.scalar.activation(
                                    dA[:], delta[:], AF.Exp,
                                    scale=Ah[:, e * DS + s: e * DS + s + 1])
                                dgB = pf.tile([128, TBH], bf16, tag="dgB",
                                              bufs=2)
                                # scans are DVE-only on HW; spread the
                                # elementwise multiplies DVE/Pool to balance
                                eng_b = (nc.vector if idx % 5 == 0
                                         else nc.gpsimd)
                                eng_b.tensor_tensor(
                                    out=dgB[:], in0=dg[:], in1=Bb[s][:],
                                    op=MUL)
                                h = pf.tile([128, TBH], bf16, tag="h", bufs=2)
                                init = (0.0 if tbh == 0 else
                                        carry[:, e * DS + s: e * DS + s + 1])
                                nc.vector.tensor_tensor_scan(
                                    h[:], dA[:], dgB[:], init,
                                    op0=MUL, op1=ADD)
                                if tbh == 0:
                                    nc.gpsimd.tensor_scalar(
                                        out=carry[:, e * DS + s:
                                                  e * DS + s + 1],
                                        in0=h[:, TBH - 1:TBH], scalar1=1.0,
                                        scalar2=None, op0=MUL)
                                ch = pf.tile([128, TBH], bf16, tag="ch",
                                             bufs=2)
                                eng_c = (nc.vector if idx % 5 == 2
                                         else nc.gpsimd)
                                eng_c.tensor_tensor(
                                    out=ch[:], in0=h[:], in1=Cb[s][:], op=MUL)
                                for j in range(2):
                                    nc.tensor.matmul(
                                        yps[j][:], Ident[:],
                                        ch[:, j * 512:(j + 1) * 512],
                                        start=False, stop=(s == DS - 1),
                                        skip_group_check=True)
                            ygt = pf.tile([128, TBH], bf16, tag=f"yg{e}",
                                          name=f"yg{e}")
                            for j in range(2):
                                nc.vector.tensor_tensor(
                                    out=ygt[:, j * 512:(j + 1) * 512],
                                    in0=yps[j][:],
                                    in1=zst[:, j * 512:(j + 1) * 512],
                                    op=MUL)
                            yg.append(ygt)
                        # out_proj for this half
                        for mb in range(8):
                            wo = pf.tile([128, NE * 128], bf16, tag="wo",
                                         bufs=2)
                            nc.sync.dma_start(wo[:], Wout_d[mb, :, :])
                            for j in range(2):
                                acco = pso.tile([128, 512], f32, tag="op",
                                                bufs=1)
                                for kb in range(NE):
                                    nc.tensor.matmul(
                                        acco[:],
                                        wo[:, kb * 128:(kb + 1) * 128],
                                        yg[kb][:, j * 512:(j + 1) * 512],
                                        start=(kb == 0), stop=(kb == NE - 1))
                                ot = pf.tile([128, 512], f32, tag="ot",
                                             bufs=2)
                                nc.vector.tensor_scalar(
                                    out=ot[:], in0=acco[:], scalar1=1.0,
                                    scalar2=None, op0=MUL)
                                nc.sync.dma_start(
                                    out_d[mb * 128:(mb + 1) * 128,
                                          hb + j * 512: hb + (j + 1) * 512],
                                    ot[:])

    nc.compile()
    return nc


def _host_inputs(inputs):
    """Build the 8 per-core input maps from the full problem inputs."""
    import ml_dtypes
    bf = ml_dtypes.bfloat16

    q = np.asarray(inputs["query"], np.float32)
    ctx = np.asarray(inputs["context"], np.float32)
    c_in_w = np.asarray(inputs["c_in_w"], np.float32)
    segc = np.asarray(inputs["seg_context"], np.float32).reshape(DM)
    segq = np.asarray(inputs["seg_query"], np.float32).reshape(DM)
    in_proj_w = np.asarray(inputs["in_proj_w"], np.float32)
    conv_w = np.asarray(inputs["conv_w"], np.float32)
    conv_b = np.asarray(inputs["conv_b"], np.float32)
    x_proj_w = np.asarray(inputs["x_proj_w"], np.float32)
    dt_proj_w = np.asarray(inputs["dt_proj_w"], np.float32)
    dt_proj_b = np.asarray(inputs["dt_proj_b"], np.float32)
    A = (-np.exp(np.asarray(inputs["A_log"], np.float32))).astype(np.float32)
    D = np.asarray(inputs["D"], np.float32)
    out_w = np.asarray(inputs["mamba_out_w"], np.float32)

    def blk(a, p=128):
        n = a.shape[0] // p
        return np.ascontiguousarray(
            a.reshape(n, p, -1).transpose(1, 0, 2).reshape(p, -1))

    # c-path composed weight [2*DI, DM] (pad DC->DM with zeros)
    Wc_comp = (in_proj_w.astype(np.float64) @ c_in_w.astype(np.float64))
    Wc_pad = np.zeros((2 * DI, DM), np.float64)
    Wc_pad[:, :DC] = Wc_comp
    bias_c = (in_proj_w.astype(np.float64) @ segc.astype(np.float64))

    def pack_W1(w):  # [2*DI, DM] -> [32, 128, DM] as [e, cin, kb*128+rout]
        return np.ascontiguousarray(
            w.reshape(2 * NE, 128, NK, 128).transpose(0, 3, 2, 1)
            .reshape(2 * NE, 128, NK * 128))

    W1_c = pack_W1(Wc_pad).astype(bf)       # c-type segment
    W1_q = pack_W1(in_proj_w.astype(np.float64)).astype(bf)  # q-type

    # fwd cores: seg0=c, seg1=q ; bwd cores: seg0=q, seg1=c
    W1_fwd = np.ascontiguousarray(np.stack([W1_c, W1_q]))
    W1_bwd = np.ascontiguousarray(np.stack([W1_q, W1_c]))

    bias_u_col = bias_c[:DI].astype(np.float32)   # [DI]
    bias_z_col = bias_c[DI:].astype(np.float32)
    zero_col = np.zeros((128, NE), np.float32)
    bU = blk(bias_u_col[:, None]).reshape(128, NE)
    bZ = blk(bias_z_col[:, None]).reshape(128, NE)
    biasU_fwd = np.ascontiguousarray(np.concatenate([bU, zero_col], 1))
    biasU_bwd = np.ascontiguousarray(np.concatenate([zero_col, bU], 1))
    biasZ_fwd = np.ascontiguousarray(np.concatenate([bZ, zero_col], 1))
    biasZ_bwd = np.ascontiguousarray(np.concatenate([zero_col, bZ], 1))

    Wxp = blk(x_proj_w.T).astype(bf)                      # [128, 16*96]
    Wdt = np.ascontiguousarray(dt_proj_w.T).astype(bf)    # [64, 2048]
    Wout = np.ascontiguousarray(
        out_w.reshape(8, 128, NE, 128).transpose(0, 3, 2, 1)
        .reshape(8, 128, NE * 128)).astype(bf)            # [8, 128, 16*128]
    convw = blk(conv_w)                                   # [128, 16*4]
    convb = conv_b.reshape(NE, 128).T.copy()
    dtb = dt_proj_b.reshape(NE, 128).T.copy()
    Ah = blk(A)                                           # [128, 16*16]
    # D as diagonal blocks [128, e*128+j]
    Dd = np.zeros((128, NE * 128), np.float32)
    for e in range(NE):
        Dd[np.arange(128), e * 128 + np.arange(128)] = D[e * 128:(e + 1) * 128]
    Dd = Dd.astype(bf)
    Ident = np.eye(128, dtype=np.float32).astype(bf)

    shared = dict(Wxp=Wxp, Wdt=Wdt, Wout=Wout, convw=convw, convb=convb,
                  dtb=dtb, Ah=Ah, Dd=Dd, Ident=Ident)

    maps = []
    for c in range(NCORE):
        d, b = divmod(c, 4)
        if d == 0:
            ctx_pad = np.zeros((DM, Lc), np.float32)
            ctx_pad[:DC] = ctx[b].T
            in1_0 = ctx_pad.astype(bf)
            in1_1 = np.ascontiguousarray((q[b] + segq).T).astype(bf)
            maps.append(dict(in1_0=in1_0, in1_1=in1_1, W1=W1_fwd,
                             biasU=biasU_fwd, biasZ=biasZ_fwd, **shared))
        else:
            in1_0 = np.ascontiguousarray((q[b][::-1] + segq).T).astype(bf)
            ctx_pad = np.zeros((DM, Lq), np.float32)
            ctx_pad[:DC] = ctx[b][::-1].T
            in1_1 = ctx_pad.astype(bf)
            maps.append(dict(in1_0=in1_0, in1_1=in1_1, W1=W1_bwd,
                             biasU=biasU_bwd, biasZ=biasZ_bwd, **shared))
    return maps


def kernel(**inputs) -> np.ndarray:
    global _prog
    from concourse.bass_utils import run_bass_kernel_spmd
    if _prog is None:
        _prog = _build()
    maps = _host_inputs(inputs)
    res = run_bass_kernel_spmd(_prog, maps, list(range(NCORE)))
    outs = [np.asarray(r["out"], np.float32) for r in res.results]
    y = np.empty((B, Lq, DM), np.float32)
    for b in range(B):
        fwd = outs[b][:, Lc:].T                    # [Lq, DM]
        bwd = outs[4 + b][:, 0:Lq][:, ::-1].T      # [Lq, DM]
        y[b] = 0.5 * (fwd + bwd)
    return y
